# revision 10
# baseline (speedup 1.0000x reference)
"""Trainium2 Bass kernel for 2-layer GAT node classification (50K nodes, 800K edges).

Strategy:
  - Host: relabel nodes by in-degree into 392 tiles of 128; tile 8r+c -> core c
    round r, so all 8 cores share an identical program structure. Edges are
    grouped per destination; each destination's in-edges are split across two
    int16-indexable gather windows of the node table and laid out as per-tile
    grids (slot lane == destination lane).
  - Device (per layer): dense phase computes T[n] = [h fp16 | al_src f32 |
    al_dst f32] (512B rows), AllGather replicates T; edge phase dma_gathers
    source rows in 4096-row batches, computes w = exp(leaky_relu(als+ald))
    (no max subtraction; magnitudes are O(1)), scales messages, accumulates
    per-destination via identity matmuls in PSUM, flushes to SBUF
    accumulators; then normalizes by the summed weights, adds bias,
    activation.  Layer 2 feeds a classifier matmul + log_softmax.
  - Outputs are written contiguously (relabeled ids); host un-permutes.
"""
import sys

sys.path.insert(0, "/opt/trn_rl_repo")

import numpy as np

import concourse.bacc as bacc
import concourse.tile as tile
import concourse.mybir as mybir
from concourse.bass_utils import run_bass_kernel_spmd

P = 128
NCORES = 8
F_IN = 128
H = 4
C = 32
HC = 128
NCLS = 40
NEG = 0.2
EPS = 1e-16
J_MAX = 8  # gather chunks per dma_gather call (SWDGE desc ring limit: 1024 rows)

f32 = mybir.dt.float32
f16 = mybir.dt.float16
u16 = mybir.dt.uint16
i16 = mybir.dt.int16

LAST_EXEC_NS = None


# ---------------------------------------------------------------------------
# host preprocessing
# ---------------------------------------------------------------------------

def _cumcount(keys):
    """rank of each element among equal keys, input sorted by keys."""
    n = len(keys)
    if n == 0:
        return np.zeros(0, dtype=np.int64)
    first = np.ones(n, dtype=bool)
    first[1:] = keys[1:] != keys[:-1]
    idx = np.arange(n)
    start = np.maximum.accumulate(np.where(first, idx, 0))
    return idx - start


def _preprocess(x, edge_index, n_real):
    """Relabel nodes, build gather grids. Returns a struct dict."""
    n_tiles = -(-(n_real + 1) // P)  # at least one pad (the lo dummy)
    n_tiles = -(-n_tiles // NCORES) * NCORES  # divisible by NCORES
    npad = n_tiles * P
    tpc = n_tiles // NCORES
    npc = tpc * P
    table_rows = npad + P  # one extra tile of rows; row npad = hi dummy
    win = 32768
    if npad <= win:
        hi_base = 0  # single window world (small test sizes)
    else:
        hi_base = table_rows - win
        assert hi_base + win >= table_rows and npad - 1 - hi_base <= 32767
    lo_top = min(win, table_rows)  # lo window covers rows [0, lo_top)

    src0 = np.asarray(edge_index[0]).astype(np.int64)
    dst0 = np.asarray(edge_index[1]).astype(np.int64)

    deg = np.bincount(dst0, minlength=npad).astype(np.int64)
    deg[:n_real] += 1  # self loops

    order = np.argsort(deg, kind="stable")  # ascending; pads (deg 0) first
    pos = np.empty(npad, dtype=np.int64)
    pos[order] = np.arange(npad)
    tile_of = pos // P
    lane_of = pos % P
    r_of = tile_of // NCORES
    c_of = tile_of % NCORES
    new_id = c_of * npc + r_of * P + lane_of  # old -> new

    assert deg[np.flatnonzero(new_id == 0)[0]] == 0, "id 0 must be a pad"

    # relabeled edge list incl self loops
    all_src = np.concatenate([new_id[src0], new_id[:n_real]])
    all_dst = np.concatenate([new_id[dst0], new_id[:n_real]])

    # sort by destination
    o = np.argsort(all_dst, kind="stable")
    s = all_src[o]
    d = all_dst[o]

    # window classification (in new-id space)
    if hi_base == 0:
        cat = np.zeros(len(s), dtype=np.int8)  # everything lo
    else:
        cat = np.full(len(s), 2, dtype=np.int8)  # flex
        cat[s < hi_base] = 0
        cat[s >= lo_top] = 1

    ndeg = np.bincount(d, minlength=npad)
    nlo = np.bincount(d[cat == 0], minlength=npad)
    nhi = np.bincount(d[cat == 1], minlength=npad)
    # balanced per-destination split
    kl_node = np.maximum(nlo, np.minimum(ndeg - nhi, (ndeg + 1) // 2))
    kh_node = ndeg - kl_node

    # flex edges: rank among flex of same dst; first (kl_node - nlo) -> lo
    flex_rank = np.zeros(len(s), dtype=np.int64)
    mflex = cat == 2
    flex_rank[mflex] = _cumcount(d[mflex])
    to_lo = (cat == 0) | (mflex & (flex_rank < (kl_node - nlo)[d]))

    # slot rank within (dst, window)
    k_slot = np.zeros(len(s), dtype=np.int64)
    for m in (to_lo, ~to_lo):
        k_slot[m] = _cumcount(d[m])

    # per-round common K values
    def round_k(k_node):
        k_tile = np.max(k_node.reshape(n_tiles, P), axis=1)  # by new-id tile
        # new-id tile index t = c*tpc + r; round r common = max over c
        return np.max(k_tile.reshape(NCORES, tpc), axis=0)  # [tpc]

    KL = round_k(kl_node)
    KH = round_k(kh_node)
    if hi_base == 0:
        KH = np.zeros_like(KH)

    cumKL = np.concatenate([[0], np.cumsum(KL)])
    cumKH = np.concatenate([[0], np.cumsum(KH)])
    CL, CH = int(cumKL[-1]), int(cumKH[-1])

    DUMMY_LO = 0
    DUMMY_HI = npad - hi_base  # table row npad

    # slot streams per core: [n_chunks*128] int16 indices
    slots_lo = np.full((NCORES, CL * P), DUMMY_LO, dtype=np.int64)
    slots_hi = np.full((NCORES, CH * P), DUMMY_HI, dtype=np.int64)

    core_e = d // npc
    r_e = (d % npc) // P
    lane_e = d % P
    pos_lo = (cumKL[r_e] + k_slot) * P + lane_e
    pos_hi = (cumKH[r_e] + k_slot) * P + lane_e
    for c in range(NCORES):
        m = (core_e == c) & to_lo
        slots_lo[c, pos_lo[m]] = s[m]
        m = (core_e == c) & ~to_lo
        slots_hi[c, pos_hi[m]] = s[m] - hi_base

    # call structure: split chunk streams into calls of <= J_MAX chunks
    def make_calls(total_chunks, cumK):
        calls = []  # (chunk_off, J, segments=[(r, j_off, j_len)])
        off = 0
        while off < total_chunks:
            jn = min(J_MAX, total_chunks - off)
            segs = []
            r0 = int(np.searchsorted(cumK, off, side="right")) - 1
            j = 0
            while j < jn:
                while int(cumK[r0 + 1]) <= off + j:
                    r0 += 1
                r_end = int(cumK[r0 + 1])
                seg_len = min(jn - j, r_end - (off + j))
                segs.append((r0, j, seg_len))
                j += seg_len
            calls.append((off, jn, segs))
            off += jn
        return calls

    calls_lo = make_calls(CL, cumKL)
    calls_hi = make_calls(CH, cumKH)

    # packed int16 index data: slot i of a call -> [16*(rep), i%16, i//16]
    def pack(slots):  # [NCORES, n_slots] -> [NCORES, 128, n_slots//16]
        ncols = slots.shape[1] // 16
        a = slots.reshape(NCORES, ncols, 16).transpose(0, 2, 1)  # [NC,16,cols]
        a = a.astype(np.uint16).view(np.int16)
        return np.tile(a, (1, 8, 1))  # replicate to 128 partitions

    idx_lo = pack(slots_lo) if CL else np.zeros((NCORES, 128, 0), np.int16)
    idx_hi = pack(slots_hi) if CH else np.zeros((NCORES, 128, 0), np.int16)
    idx_all = np.concatenate([idx_lo, idx_hi], axis=2)
    idx_all = np.ascontiguousarray(idx_all)
    lo_cols = idx_lo.shape[2]

    # padded, permuted, transposed x
    x = np.asarray(x, dtype=np.float32)
    x_pad = np.zeros((npad, x.shape[1]), dtype=np.float32)
    x_pad[new_id[:n_real]] = x
    xT = np.ascontiguousarray(
        x_pad.reshape(NCORES, npc, x.shape[1]).transpose(0, 2, 1)
    )  # [NCORES, F, npc]

    return dict(
        npad=npad, npc=npc, tpc=tpc, table_rows=table_rows,
        hi_base=hi_base, lo_top=lo_top,
        KL=KL.astype(int), KH=KH.astype(int),
        calls_lo=calls_lo, calls_hi=calls_hi,
        idx_all=idx_all, lo_cols=lo_cols,
        xT=xT, new_id=new_id, n_real=n_real,
        slots_lo=slots_lo, slots_hi=slots_hi, cumKL=cumKL, cumKH=cumKH,
    )


def _wfull(W, a_src, a_dst):
    W = np.asarray(W, dtype=np.float32)
    fin = W.shape[0]
    Wf = W.reshape(fin, HC)
    Was = np.zeros((HC, H), dtype=np.float32)
    Wad = np.zeros((HC, H), dtype=np.float32)
    for h in range(H):
        Was[h * C:(h + 1) * C, h] = np.asarray(a_src, np.float32)[h]
        Wad[h * C:(h + 1) * C, h] = np.asarray(a_dst, np.float32)[h]
    return np.ascontiguousarray(
        np.concatenate([Wf, Wf @ Was, Wf @ Wad], axis=1)
    )  # [fin, 136]


def _dummy_row():
    row = np.zeros(256, dtype=np.uint16)
    fpart = np.array([-1e30] * 4 + [0.0] * 4, dtype=np.float32)
    row[128:144] = fpart.view(np.uint16)
    return row[None, :]


# ---------------------------------------------------------------------------
# device program
# ---------------------------------------------------------------------------

def _build(st):
    npc, tpc = st["npc"], st["tpc"]
    table_rows = st["table_rows"]
    hi_base, lo_top = st["hi_base"], st["lo_top"]
    KL, KH = st["KL"], st["KH"]
    calls_lo, calls_hi = st["calls_lo"], st["calls_hi"]
    tot_cols = st["idx_all"].shape[2]
    lo_cols = st["lo_cols"]
    npad = st["npad"]

    nc = bacc.Bacc(None, target_bir_lowering=False)

    xT_in = nc.dram_tensor("xT", [F_IN, npc], f32, kind="ExternalInput")
    idx_in = nc.dram_tensor("idx_all", [128, max(tot_cols, 16)], i16, kind="ExternalInput")
    wfull1_in = nc.dram_tensor("wfull1", [F_IN, 136], f32, kind="ExternalInput")
    wfull2_in = nc.dram_tensor("wfull2", [HC, 136], f32, kind="ExternalInput")
    wc_in = nc.dram_tensor("wc", [HC, NCLS], f32, kind="ExternalInput")
    b1_in = nc.dram_tensor("b1", [1, HC], f32, kind="ExternalInput")
    b2_in = nc.dram_tensor("b2", [1, HC], f32, kind="ExternalInput")
    bc_in = nc.dram_tensor("bc", [1, NCLS], f32, kind="ExternalInput")
    dummy_in = nc.dram_tensor("dummyrow", [1, 256], u16, kind="ExternalInput")
    ident16_in = nc.dram_tensor("ident16", [P, P], f16, kind="ExternalInput")
    ident32_in = nc.dram_tensor("ident32", [P, P], f32, kind="ExternalInput")

    logits_out = nc.dram_tensor("logits", [npc, NCLS], f32, kind="ExternalOutput")

    t_local = nc.dram_tensor("t_local", [npc, 256], u16)
    t_full = nc.dram_tensor("t_full", [table_rows, 256], u16, addr_space="Shared")
    x2t_dram = nc.dram_tensor("x2t", [HC, npc], f32)

    rg = [list(range(NCORES))]

    with tile.TileContext(nc) as tc:
        with (
            tc.tile_pool(name="const", bufs=1) as constp,
            tc.tile_pool(name="xt", bufs=1) as xtp,
            tc.tile_pool(name="wf", bufs=2) as wfp,
            tc.tile_pool(name="ald", bufs=2) as aldp,
            tc.tile_pool(name="tt", bufs=3) as ttp,
            tc.tile_pool(name="g", bufs=2) as gp,
            tc.tile_pool(name="m", bufs=2) as mp,
            tc.tile_pool(name="w32", bufs=2) as wp,
            tc.tile_pool(name="acc", bufs=tpc) as accp,
            tc.tile_pool(name="norm", bufs=3) as normp,
            tc.tile_pool(name="smalls", bufs=4) as smallp,
            tc.tile_pool(name="psd", bufs=2, space="PSUM") as psd,
            tc.tile_pool(name="pse", bufs=3, space="PSUM") as pse,
            tc.tile_pool(name="pst", bufs=2, space="PSUM") as pstp,
            tc.tile_pool(name="psc", bufs=1, space="PSUM") as pscp,
        ):
            # constants
            ident16 = constp.tile([P, P], f16)
            nc.sync.dma_start(out=ident16[:, :], in_=ident16_in[:, :])
            ident32 = constp.tile([P, P], f32)
            nc.sync.dma_start(out=ident32[:, :], in_=ident32_in[:, :])
            wc_sb = constp.tile([HC, NCLS], f32)
            nc.sync.dma_start(out=wc_sb[:, :], in_=wc_in[:, :])
            dummy_sb = constp.tile([1, 256], u16)
            nc.sync.dma_start(out=dummy_sb[:, :], in_=dummy_in[:, :])
            idx_sb = constp.tile([128, max(tot_cols, 16)], i16)
            nc.sync.dma_start(out=idx_sb[:, :], in_=idx_in[:, :])
            b_sb = {}
            for name, hnd, w in (("b1", b1_in, HC), ("b2", b2_in, HC), ("bc", bc_in, NCLS)):
                t = constp.tile([P, w], f32, name=f"bsb_{name}", tag=f"bsb_{name}")
                nc.sync.dma_start(out=t[:, :], in_=hnd[0:1, :].to_broadcast((P, w)))
                b_sb[name] = t
            # write hi-dummy table row once (outside AG range)
            if hi_base:
                nc.sync.dma_start(out=t_full[npad:npad + 1, :], in_=dummy_sb[:, :])

            for layer in (1, 2):
                wf_sb = wfp.tile([F_IN, 136], f32)
                nc.sync.dma_start(
                    out=wf_sb[:, :], in_=(wfull1_in if layer == 1 else wfull2_in)[:, :]
                )
                xt_sb = xtp.tile([F_IN, npc], f32, tag="xt")
                nc.sync.dma_start(
                    out=xt_sb[:, :],
                    in_=(xT_in[:, :] if layer == 1 else x2t_dram[:, :]),
                )

                # ---- dense phase: node table ----
                for r in range(tpc):
                    ps = psd.tile([P, 136], f32)
                    nc.tensor.matmul(
                        out=ps[:, :],
                        lhsT=xt_sb[:, r * P:(r + 1) * P],
                        rhs=wf_sb[:, :],
                        start=True, stop=True,
                    )
                    tt = ttp.tile([P, 256], u16, tag="tt")
                    nc.vector.memset(tt[:, 144:256], 0)
                    nc.vector.tensor_copy(
                        out=tt[:, 0:128].bitcast(f16), in_=ps[:, 0:128]
                    )
                    nc.vector.tensor_copy(
                        out=tt[:, 128:144].bitcast(f32), in_=ps[:, 128:136]
                    )
                    nc.sync.dma_start(
                        out=t_local[r * P:(r + 1) * P, :], in_=tt[:, :]
                    )

                # ald prefetch from local table (own rows == own dst tiles)
                ald_sb = aldp.tile([P, tpc, 8], u16, tag="ald")
                nc.sync.dma_start(
                    out=ald_sb[:, :, :],
                    in_=t_local[:, 136:144].rearrange("(r l) w -> l r w", l=P),
                )

                # ---- AllGather the table ----
                nc.gpsimd.collective_compute(
                    "AllGather",
                    mybir.AluOpType.bypass,
                    ins=[t_local[:, :]],
                    outs=[t_full[0:npad, :]],
                    replica_groups=rg,
                )
                # lo-dummy row (row 0) must have als = -1e30
                nc.sync.dma_start(out=t_full[0:1, :], in_=dummy_sb[:, :])

                # ---- edge phase ----
                acc = [accp.tile([P, 132], f32, tag="acc", name=f"acc{i}")
                       for i in range(tpc)]
                for a in acc:
                    nc.vector.memset(a[:, :], 0.0)

                ald_f32 = ald_sb[:, :, :].bitcast(f32)  # [P, tpc, 4]

                for wname, calls, col0, base, wtop in (
                    ("lo", calls_lo, 0, 0, lo_top),
                    ("hi", calls_hi, lo_cols, hi_base, table_rows),
                ):
                    if base == 0 and wname == "hi":
                        continue
                    win_rows = wtop - base
                    for (chunk_off, jn, segs) in calls:
                        g = gp.tile([P, jn, 256], u16, tag="g")
                        nc.gpsimd.dma_gather(
                            out_ap=g[:, :, :],
                            in_ap=t_full[base:base + win_rows, :],
                            idxs_ap=idx_sb[:, col0 + chunk_off * 8:
                                           col0 + (chunk_off + jn) * 8],
                            num_idxs=jn * P,
                            num_idxs_reg=jn * P,
                            elem_size=256,
                        )
                        w32 = wp.tile([P, jn, 4], f32, tag="w32")
                        for (r, j0, jl) in segs:
                            nc.vector.tensor_tensor(
                                out=w32[:, j0:j0 + jl, :],
                                in0=g[:, j0:j0 + jl, 128:136].bitcast(f32),
                                in1=ald_f32[:, r:r + 1, :].to_broadcast((P, jl, 4)),
                                op=mybir.AluOpType.add,
                            )
                        wtmp = wp.tile([P, jn, 4], f32, tag="wtmp")
                        nc.vector.tensor_scalar_mul(
                            out=wtmp[:, :, :], in0=w32[:, :, :], scalar1=NEG
                        )
                        nc.vector.tensor_tensor(
                            out=w32[:, :, :], in0=w32[:, :, :], in1=wtmp[:, :, :],
                            op=mybir.AluOpType.max,
                        )
                        nc.scalar.activation(
                            out=w32[:, :, :], in_=w32[:, :, :],
                            func=mybir.ActivationFunctionType.Exp,
                        )
                        m = mp.tile([P, jn, 132], f16, tag="m")
                        nc.vector.tensor_copy(out=m[:, :, 128:132], in_=w32[:, :, :])
                        nc.vector.tensor_tensor(
                            out=m[:, :, 0:128].rearrange("p j (h c) -> p j h c", h=H),
                            in0=g[:, :, 0:128].bitcast(f16).rearrange(
                                "p j (h c) -> p j h c", h=H),
                            in1=m[:, :, 128:132][:, :, :, None].to_broadcast(
                                (P, jn, H, C)),
                            op=mybir.AluOpType.mult,
                        )
                        for (r, j0, jl) in segs:
                            ps = pse.tile([P, 132], f32, tag="pse")
                            for j in range(j0, j0 + jl):
                                nc.tensor.matmul(
                                    out=ps[:, :],
                                    lhsT=ident16[:, :],
                                    rhs=m[:, j, :],
                                    start=(j == j0), stop=(j == j0 + jl - 1),
                                )
                            nc.vector.tensor_tensor(
                                out=acc[r][:, :], in0=acc[r][:, :], in1=ps[:, :],
                                op=mybir.AluOpType.add,
                            )

                # ---- normalize + activation + tail ----
                bias = b_sb["b1"] if layer == 1 else b_sb["b2"]
                for r in range(tpc):
                    recip = smallp.tile([P, 4], f32, tag="recip")
                    nc.vector.tensor_scalar_add(
                        out=recip[:, :], in0=acc[r][:, 128:132], scalar1=EPS
                    )
                    nc.vector.reciprocal(out=recip[:, :], in_=recip[:, :])
                    xn = normp.tile([P, HC], f32, tag="xn")
                    nc.vector.tensor_tensor(
                        out=xn[:, :].rearrange("p (h c) -> p h c", h=H),
                        in0=acc[r][:, 0:128].rearrange("p (h c) -> p h c", h=H),
                        in1=recip[:, :, None].to_broadcast((P, H, C)),
                        op=mybir.AluOpType.mult,
                    )
                    nc.vector.tensor_tensor(
                        out=xn[:, :], in0=xn[:, :], in1=bias[:, :],
                        op=mybir.AluOpType.add,
                    )
                    xtmp = normp.tile([P, HC], f32, tag="xtmp")
                    nc.vector.tensor_scalar_mul(
                        out=xtmp[:, :], in0=xn[:, :], scalar1=NEG
                    )
                    nc.vector.tensor_tensor(
                        out=xn[:, :], in0=xn[:, :], in1=xtmp[:, :],
                        op=mybir.AluOpType.max,
                    )
                    pt = pstp.tile([P, P], f32, tag="pt")
                    nc.tensor.transpose(
                        out=pt[:, :], in_=xn[:, :], identity=ident32[:, :]
                    )
                    xt2 = ttp.tile([P, P], f32, tag="xt2")
                    nc.vector.tensor_copy(out=xt2[:, :], in_=pt[:, :])
                    if layer == 1:
                        nc.sync.dma_start(
                            out=x2t_dram[:, r * P:(r + 1) * P], in_=xt2[:, :]
                        )
                    else:
                        pc = pscp.tile([P, NCLS], f32, tag="pc")
                        nc.tensor.matmul(
                            out=pc[:, :], lhsT=xt2[:, :], rhs=wc_sb[:, :],
                            start=True, stop=True,
                        )
                        lg = normp.tile([P, NCLS], f32, tag="lg")
                        nc.vector.tensor_tensor(
                            out=lg[:, :], in0=pc[:, :], in1=b_sb["bc"][:, :],
                            op=mybir.AluOpType.add,
                        )
                        mx = smallp.tile([P, 1], f32, tag="mx")
                        nc.vector.reduce_max(
                            out=mx[:, :], in_=lg[:, :], axis=mybir.AxisListType.X
                        )
                        zs = normp.tile([P, NCLS], f32, tag="zs")
                        nc.vector.tensor_scalar(
                            out=zs[:, :], in0=lg[:, :], scalar1=mx[:, :],
                            scalar2=None, op0=mybir.AluOpType.subtract,
                        )
                        es = normp.tile([P, NCLS], f32, tag="es")
                        nc.scalar.activation(
                            out=es[:, :], in_=zs[:, :],
                            func=mybir.ActivationFunctionType.Exp,
                        )
                        sm = smallp.tile([P, 1], f32, tag="sm")
                        nc.vector.reduce_sum(
                            out=sm[:, :], in_=es[:, :], axis=mybir.AxisListType.X
                        )
                        ls = smallp.tile([P, 1], f32, tag="ls")
                        nc.scalar.activation(
                            out=ls[:, :], in_=sm[:, :],
                            func=mybir.ActivationFunctionType.Ln,
                        )
                        ot = normp.tile([P, NCLS], f32, tag="ot")
                        nc.vector.tensor_scalar(
                            out=ot[:, :], in0=zs[:, :], scalar1=ls[:, :],
                            scalar2=None, op0=mybir.AluOpType.subtract,
                        )
                        nc.sync.dma_start(
                            out=logits_out[r * P:(r + 1) * P, :], in_=ot[:, :]
                        )

    nc.finalize()
    return nc


# ---------------------------------------------------------------------------
# entry point
# ---------------------------------------------------------------------------

_CACHE = {}


def kernel(x, edge_index, W1, a1_src, a1_dst, b1, W2, a2_src, a2_dst, b2, Wc, bc):
    global LAST_EXEC_NS
    import os

    x = np.asarray(x, dtype=np.float32)
    n_real = x.shape[0]
    ekey = hash((n_real,) + tuple(np.asarray(edge_index[0][:16]).tolist())
                ) ^ hash(np.asarray(edge_index).tobytes())
    if ekey in _CACHE:
        nc, st = _CACHE[ekey]
    else:
        st = _preprocess(x, edge_index, n_real)
        nc = _build(st)
        _CACHE[ekey] = (nc, st)
        st["xT"] = st["xT"]  # keep
    # per-run inputs (x could differ even with same edges; recompute xT)
    st2 = st
    npad, npc = st2["npad"], st2["npc"]
    new_id = st2["new_id"]
    x_pad = np.zeros((npad, F_IN), dtype=np.float32)
    x_pad[new_id[:n_real]] = x
    xT = np.ascontiguousarray(
        x_pad.reshape(NCORES, npc, F_IN).transpose(0, 2, 1))

    wfull1 = _wfull(W1, a1_src, a1_dst)
    wfull2 = _wfull(W2, a2_src, a2_dst)
    wc = np.ascontiguousarray(np.asarray(Wc, dtype=np.float32))
    b1r = np.asarray(b1, dtype=np.float32)[None, :]
    b2r = np.asarray(b2, dtype=np.float32)[None, :]
    bcr = np.asarray(bc, dtype=np.float32)[None, :]
    dummy = _dummy_row()
    ident16 = np.eye(P, dtype=np.float16)
    ident32 = np.eye(P, dtype=np.float32)

    idx_all = st2["idx_all"]
    if idx_all.shape[2] == 0:
        idx_all = np.zeros((NCORES, 128, 16), np.int16)
    elif idx_all.shape[2] < 16:
        pad = np.zeros((NCORES, 128, 16 - idx_all.shape[2]), np.int16)
        idx_all = np.concatenate([idx_all, pad], axis=2)

    in_maps = []
    for c in range(NCORES):
        in_maps.append({
            "xT": xT[c],
            "idx_all": np.ascontiguousarray(idx_all[c]),
            "wfull1": wfull1, "wfull2": wfull2, "wc": wc,
            "b1": b1r, "b2": b2r, "bc": bcr,
            "dummyrow": dummy, "ident16": ident16, "ident32": ident32,
        })

    os.environ.setdefault("BASS_NEVER_TRACE", "1")  # no NTFF hook in this env
    res = run_bass_kernel_spmd(nc, in_maps, core_ids=list(range(NCORES)))
    LAST_EXEC_NS = res.exec_time_ns

    logits_pad = np.concatenate([res.results[c]["logits"] for c in range(NCORES)], axis=0)
    return logits_pad[new_id[:n_real]].astype(np.float32)
